# revision 18
# baseline (speedup 1.0000x reference)
"""MLA (multi-head latent attention) Bass kernel for Trainium2, 8 NeuronCores.

Sharding: core c -> batch b=c//4, head group hg=c%4 (4 heads each), plus
sequence-parallel down-projections (seq chunk sc=c%4, 512 rows) with on-device
AllGather of the low-rank latents. Final output projection produces per-core
partial sums over its 4 heads; the host sums the 4 partials per batch.

Host-side prep (sharding/layout only): transpose x chunk and weights so the
contraction dim lands on SBUF partitions, fold RMS-norm weights and the
attention scale into the up-projection weights, extract diagonal mask blocks.
"""
import math
import numpy as np
from contextlib import ExitStack

import concourse.bass as bass
import concourse.tile as tile
from concourse import mybir, bacc
from concourse.bass_utils import run_bass_kernel_spmd

# Problem constants (hardcoded per contract)
B, S, D, H = 2, 2048, 2048, 16
Q_LORA, KV_LORA = 1536, 512
D_NOPE, D_ROPE, D_V = 128, 64, 128
QK_D = D_NOPE + D_ROPE  # 192
HDV = 4 * D_V  # per-core head-group output dim (512)
EPS = 1e-6
N_CORES = 8
SC = S // 4  # seq chunk per core within a batch group (512)
F32 = mybir.dt.float32
F32R = mybir.dt.float32r

_cache = {}
last_exec_time_ns = None
last_results = None


def _r(ap):
    return ap.bitcast(F32R)


def _bcast_free(ap, n):
    """[P,1] AP -> [P,n] free-dim 0-stride broadcast view."""
    return bass.AP(tensor=ap.tensor, offset=ap.offset, ap=[ap.ap[0], [0, n]])


def _build(causal: bool):
    nc = bacc.Bacc(trn_type="TRN2", target_bir_lowering=False, debug=False,
                   num_devices=N_CORES)

    def din(name, shape):
        return nc.dram_tensor(name, shape, F32, kind="ExternalInput").ap()

    xT = din("xT", [D, SC])
    wqaT = din("wqaT", [D, Q_LORA])
    wkvaT = din("wkvaT", [D, KV_LORA + D_ROPE])
    wqbT = din("wqbT", [Q_LORA, 4 * QK_D])
    wkvbT = din("wkvbT", [KV_LORA, 4 * (D_NOPE + D_V)])
    woT = din("woT", [HDV, D])
    cos4 = din("cos4", [S, 4 * (D_ROPE // 2)])  # per-head-replicated cos, [S,128]
    sin4 = din("sin4", [S, 4 * (D_ROPE // 2)])
    cosk = din("cosk", [SC, D_ROPE // 2])  # this core's seq chunk rows
    sink = din("sink", [SC, D_ROPE // 2])
    if causal:
        maskd = din("maskd", [16, 128, 128])  # diagonal blocks of mask
    else:
        maskf = din("maskf", [S, S])
    out = nc.dram_tensor("out", [S, D], F32, kind="ExternalOutput").ap()

    kv_stage = nc.dram_tensor("kv_stage", [KV_LORA + D_ROPE, SC], F32).ap()
    kv_gather = nc.dram_tensor("kv_gather", [4, KV_LORA + D_ROPE, SC], F32).ap()
    cq_stage = nc.dram_tensor("cq_stage", [Q_LORA, SC], F32).ap()
    cq_gather = nc.dram_tensor("cq_gather", [4, Q_LORA, SC], F32).ap()
    GROUPS = [[0, 1, 2, 3], [4, 5, 6, 7]]

    with tile.TileContext(nc) as tc, ExitStack() as top:
        persist = top.enter_context(tc.tile_pool(name="persist", bufs=1))
        ident0 = persist.tile([128, 128], F32)
        nc.gpsimd.memset(ident0[:], 0.0)
        nc.gpsimd.affine_select(
            out=ident0[:], in_=ident0[:],
            compare_op=mybir.AluOpType.not_equal, fill=1.0,
            base=0, pattern=[[-1, 128]], channel_multiplier=1)
        ident = persist.tile([128, 128], F32)
        nc.vector.tensor_copy(out=_r(ident[:]), in_=ident0[:])
        eps_sb = persist.tile([128, 1], F32)
        nc.vector.memset(eps_sb, EPS)
        zero_sb = persist.tile([128, 128], F32)
        nc.vector.memset(zero_sb, 0.0)

        psB = top.enter_context(tc.tile_pool(name="psB", bufs=2, space="PSUM"))
        psT = top.enter_context(tc.tile_pool(name="psT", bufs=2, space="PSUM"))

        def rms_norm(out_ap, in_ap, ddim, tmp_pool):
            sq = tmp_pool.tile([128, ddim], F32)
            nc.vector.tensor_mul(sq, in_ap, in_ap)
            ss = tmp_pool.tile([128, 1], F32)
            nc.vector.tensor_reduce(ss, sq, mybir.AxisListType.X, mybir.AluOpType.add)
            std = tmp_pool.tile([128, 1], F32)
            nc.scalar.activation(std, ss, mybir.ActivationFunctionType.Sqrt,
                                 bias=eps_sb, scale=1.0 / ddim)
            rstd = tmp_pool.tile([128, 1], F32)
            nc.vector.reciprocal(rstd, std)
            nc.scalar.mul(_r(out_ap), in_ap, rstd)

        def rope(out3, in3, cos_ap, sin_ap, nh, tmp_pool):
            # in3/out3: [128, nh, 64] views (pairs interleaved in last dim);
            # cos/sin: [128, nh*32] contiguous tiles. Safe for out3 == in3.
            def iv(a3, par):  # [128, nh, 32] view of pair element par
                r2 = a3.rearrange("p h (d two) -> p h d two", two=2)
                return r2[:, :, :, par]
            c3 = cos_ap.rearrange("p (h d) -> p h d", h=nh)
            s3 = sin_ap.rearrange("p (h d) -> p h d", h=nh)
            xr, xi = iv(in3, 0), iv(in3, 1)
            t1 = tmp_pool.tile([128, nh, 32], F32)
            t2 = tmp_pool.tile([128, nh, 32], F32)
            t3 = tmp_pool.tile([128, nh, 32], F32)
            t4 = tmp_pool.tile([128, nh, 32], F32)
            nc.vector.tensor_mul(t1, xr, c3)
            nc.vector.tensor_mul(t2, xi, s3)
            nc.vector.tensor_mul(t3, xr, s3)
            nc.vector.tensor_mul(t4, xi, c3)
            nc.vector.tensor_sub(_r(iv(out3, 0)), t1, t2)
            nc.vector.tensor_add(_r(iv(out3, 1)), t3, t4)

        def transpose_to(dst_ap, src_ap, rhs=None, dt_r=True, copy_eng=None):
            # PE transpose src [p,f] -> psum [f,p] (f32r), copy into dst_ap
            f = src_ap.shape[1]
            ps = psT.tile([128, 128], F32, name="ps")
            nc.tensor.matmul(_r(ps[:f, :src_ap.shape[0]]), _r(src_ap),
                             _r(ident[:] if rhs is None else rhs),
                             is_transpose=True)
            eng = copy_eng or nc.vector
            if eng is nc.scalar:
                eng.copy(_r(dst_ap), _r(ps[:f, :src_ap.shape[0]]))
            else:
                eng.tensor_copy(out=_r(dst_ap), in_=_r(ps[:f, :src_ap.shape[0]]))

        # ---------------- Phase A: load xT ----------------
        xT_pool = tc.alloc_tile_pool(name="xT", bufs=1)
        xT_sb = []
        for k in range(16):
            t = xT_pool.tile([128, SC], F32, name=f"xT{k}")
            nc.sync.dma_start(_r(t[:]), _r(xT[k * 128:(k + 1) * 128, :]))
            xT_sb.append(t)

        # ---------------- Phase B: kv down-proj + norm + rope + T + AG ----
        with ExitStack() as phB:
            wpool = phB.enter_context(tc.tile_pool(name="wkva", bufs=2))
            kvf_pool = phB.enter_context(tc.tile_pool(name="kvf", bufs=1))
            tmp = phB.enter_context(tc.tile_pool(name="tmpB", bufs=4))
            stg = phB.enter_context(tc.tile_pool(name="stgB", bufs=4))
            kvf_sb = [kvf_pool.tile([128, KV_LORA + D_ROPE], F32, name=f"kvf{i}") for i in range(4)]
            for (n0, nw) in [(0, 288), (288, 288)]:
                wk = [wpool.tile([128, nw], F32, name=f"wkva_{k}") for k in range(16)]
                for k in range(16):
                    nc.sync.dma_start(_r(wk[k][:]), _r(wkvaT[k * 128:(k + 1) * 128, n0:n0 + nw]))
                for stl in range(4):
                    ps = psB.tile([128, 512], F32, name="ps")
                    for k in range(16):
                        nc.tensor.matmul(ps[:, :nw], _r(xT_sb[k][:, stl * 128:(stl + 1) * 128]),
                                         _r(wk[k][:]), start=(k == 0), stop=(k == 15))
                    eng = nc.vector if stl % 2 == 0 else nc.scalar
                    if eng is nc.vector:
                        nc.vector.tensor_copy(out=_r(kvf_sb[stl][:, n0:n0 + nw]), in_=ps[:, :nw])
                    else:
                        nc.scalar.copy(_r(kvf_sb[stl][:, n0:n0 + nw]), ps[:, :nw])
            for stl in range(4):
                rms_norm(kvf_sb[stl][:, :KV_LORA], kvf_sb[stl][:, :KV_LORA], KV_LORA, tmp)
                ck = tmp.tile([128, 32], F32)
                sk = tmp.tile([128, 32], F32)
                nc.sync.dma_start(ck[:], cosk[stl * 128:(stl + 1) * 128, :])
                nc.sync.dma_start(sk[:], sink[stl * 128:(stl + 1) * 128, :])
                kpe = tmp.tile([128, D_ROPE], F32)
                rope(kpe[:].rearrange("p (h d) -> p h d", h=1),
                     kvf_sb[stl][:, KV_LORA:].rearrange("p (h d) -> p h d", h=1),
                     ck[:], sk[:], 1, tmp)
                for dt_ in range(4):
                    blk = stg.tile([128, 128], F32)
                    transpose_to(blk[:], kvf_sb[stl][:, dt_ * 128:(dt_ + 1) * 128])
                    nc.gpsimd.dma_start(
                        kv_stage[dt_ * 128:(dt_ + 1) * 128, stl * 128:(stl + 1) * 128], blk[:])
                blk = stg.tile([64, 128], F32)
                transpose_to(blk[:], kpe[:])
                nc.gpsimd.dma_start(
                    kv_stage[KV_LORA:, stl * 128:(stl + 1) * 128], blk[:])
            nc.gpsimd.collective_compute(
                "AllGather", mybir.AluOpType.bypass, replica_groups=GROUPS,
                ins=[kv_stage[:]], outs=[kv_gather[:]])

        # ---------------- Phase C: cq down-proj + norm + T + AG ----------
        with ExitStack() as phC:
            wpool = phC.enter_context(tc.tile_pool(name="wqa", bufs=2))
            cq_pool = phC.enter_context(tc.tile_pool(name="cq", bufs=1))
            tmp = phC.enter_context(tc.tile_pool(name="tmpC", bufs=4))
            stg = phC.enter_context(tc.tile_pool(name="stgC", bufs=4))
            cq_sb = [cq_pool.tile([128, Q_LORA], F32, name=f"cqsb{i}") for i in range(4)]
            for ci in range(3):
                n0 = ci * 512
                wk = [wpool.tile([128, 512], F32, name=f"wqa_{k}") for k in range(16)]
                for k in range(16):
                    nc.sync.dma_start(_r(wk[k][:]), _r(wqaT[k * 128:(k + 1) * 128, n0:n0 + 512]))
                for stl in range(4):
                    ps = psB.tile([128, 512], F32, name="ps")
                    for k in range(16):
                        nc.tensor.matmul(ps[:], _r(xT_sb[k][:, stl * 128:(stl + 1) * 128]),
                                         _r(wk[k][:]), start=(k == 0), stop=(k == 15))
                    if stl % 2 == 0:
                        nc.vector.tensor_copy(out=_r(cq_sb[stl][:, n0:n0 + 512]), in_=ps[:])
                    else:
                        nc.scalar.copy(_r(cq_sb[stl][:, n0:n0 + 512]), ps[:])
            for stl in range(4):
                rms_norm(cq_sb[stl][:], cq_sb[stl][:], Q_LORA, tmp)
                for dt_ in range(12):
                    blk = stg.tile([128, 128], F32)
                    transpose_to(blk[:], cq_sb[stl][:, dt_ * 128:(dt_ + 1) * 128])
                    nc.gpsimd.dma_start(
                        cq_stage[dt_ * 128:(dt_ + 1) * 128, stl * 128:(stl + 1) * 128], blk[:])
            nc.gpsimd.collective_compute(
                "AllGather", mybir.AluOpType.bypass, replica_groups=GROUPS,
                ins=[cq_stage[:]], outs=[cq_gather[:]])
        xT_pool.release()

        # ---------------- Phase D: kv up-proj (full S, this head group) ---
        kvu_pool = tc.alloc_tile_pool(name="kvu", bufs=1, side="right")
        kvu_sb = [kvu_pool.tile([128, 1024], F32, name=f"kvu{st}") for st in range(16)]
        with ExitStack() as phD:
            wpool = phD.enter_context(tc.tile_pool(name="wkvb", bufs=1))
            lpool = phD.enter_context(tc.tile_pool(name="kvl", bufs=3))
            wb = [wpool.tile([128, 1024], F32, name=f"wkvb{k}") for k in range(4)]
            for k in range(4):
                nc.sync.dma_start(_r(wb[k][:]), _r(wkvbT[k * 128:(k + 1) * 128, :]))
            for st in range(16):
                g, stl = st // 4, st % 4
                lk = [lpool.tile([128, 128], F32, name=f"kvlk{k}") for k in range(4)]
                for k in range(4):
                    nc.sync.dma_start(
                        _r(lk[k][:]), _r(kv_gather[g, k * 128:(k + 1) * 128,
                                                   stl * 128:(stl + 1) * 128]))
                for ncho in range(2):
                    ps = psB.tile([128, 512], F32, name="ps")
                    for k in range(4):
                        nc.tensor.matmul(ps[:], _r(lk[k][:]),
                                         _r(wb[k][:, ncho * 512:(ncho + 1) * 512]),
                                         start=(k == 0), stop=(k == 3))
                    if (st + ncho) % 2 == 0:
                        nc.vector.tensor_copy(out=_r(kvu_sb[st][:, ncho * 512:(ncho + 1) * 512]), in_=ps[:])
                    else:
                        nc.scalar.copy(_r(kvu_sb[st][:, ncho * 512:(ncho + 1) * 512]), ps[:])

        # ---------------- Phase E: q up-proj + rope + qT ------------------
        qT_pool = tc.alloc_tile_pool(name="qT", bufs=1, side="right")
        qT1 = [qT_pool.tile([128, S], F32, name=f"qT1_{h}") for h in range(4)]
        qT2 = [qT_pool.tile([64, S], F32, name=f"qT2_{h}") for h in range(4)]
        with ExitStack() as phE:
            wpool = phE.enter_context(tc.tile_pool(name="wqb", bufs=1))
            lpool = phE.enter_context(tc.tile_pool(name="cql", bufs=2))
            qpool = phE.enter_context(tc.tile_pool(name="qsb", bufs=3))
            tmp = phE.enter_context(tc.tile_pool(name="tmpE", bufs=4))
            wb = [wpool.tile([128, 768], F32, name=f"wqb{k}") for k in range(12)]
            for k in range(12):
                nc.sync.dma_start(_r(wb[k][:]), _r(wqbT[k * 128:(k + 1) * 128, :]))
            for st in range(16):
                g, stl = st // 4, st % 4
                lk = [lpool.tile([128, 128], F32, name=f"cqlk{k}") for k in range(12)]
                for k in range(12):
                    nc.sync.dma_start(
                        _r(lk[k][:]), _r(cq_gather[g, k * 128:(k + 1) * 128,
                                                   stl * 128:(stl + 1) * 128]))
                q_sb = qpool.tile([128, 768], F32)
                for (n0, nw) in [(0, 512), (512, 256)]:
                    ps = psB.tile([128, 512], F32, name="ps")
                    for k in range(12):
                        nc.tensor.matmul(ps[:, :nw], _r(lk[k][:]),
                                         _r(wb[k][:, n0:n0 + nw]),
                                         start=(k == 0), stop=(k == 11))
                    if n0 == 0:
                        nc.vector.tensor_copy(out=_r(q_sb[:, :512]), in_=ps[:, :512])
                    else:
                        nc.scalar.copy(_r(q_sb[:, 512:]), ps[:, :256])
                c4 = tmp.tile([128, 128], F32)
                s4 = tmp.tile([128, 128], F32)
                nc.sync.dma_start(c4[:], cos4[st * 128:(st + 1) * 128, :])
                nc.sync.dma_start(s4[:], sin4[st * 128:(st + 1) * 128, :])
                # rope the pe sub-blocks of the 4 heads: cols h*192+128 .. +64
                qpe = q_sb[:].rearrange("p (h d) -> p h d", h=4)[:, :, D_NOPE:]
                rope(qpe, qpe, c4[:], s4[:], 4, tmp)
                for hh in range(4):
                    transpose_to(qT1[hh][:, st * 128:(st + 1) * 128],
                                 q_sb[:, hh * 192:hh * 192 + 128])
                    transpose_to(qT2[hh][:, st * 128:(st + 1) * 128],
                                 q_sb[:, hh * 192 + 128:hh * 192 + 192])

        # ---------------- Phase F: attention per head ---------------------
        attn_pool = tc.alloc_tile_pool(name="attnT", bufs=1)
        attnT = [attn_pool.tile([128, S], F32, name=f"attnT{h}") for h in range(4)]
        with ExitStack() as phF:
            kpool = phF.enter_context(tc.tile_pool(name="knT", bufs=1))
            ppool = phF.enter_context(tc.tile_pool(name="probs", bufs=1))
            ptpool = phF.enter_context(tc.tile_pool(name="probsT", bufs=1))
            spool = phF.enter_context(tc.tile_pool(name="smallF", bufs=8))
            mpool = phF.enter_context(tc.tile_pool(name="maskp", bufs=1 if causal else 6))
            psS = phF.enter_context(tc.tile_pool(name="psS", bufs=2, space="PSUM"))
            psO = phF.enter_context(tc.tile_pool(name="psO", bufs=1, space="PSUM"))
            kpeT = kpool.tile([64, S], F32)
            for g in range(4):
                nc.sync.dma_start(_r(kpeT[:, g * 512:(g + 1) * 512]),
                                  _r(kv_gather[g, KV_LORA:, :]))
            if causal:
                # all 16 diagonal blocks of a causal mask are identical
                md_sb = mpool.tile([128, 128], F32, name="md0")
                nc.sync.dma_start(md_sb[:], maskd[0])
            knT = kpool.tile([128, S], F32)
            for h in range(4):
                for st in range(16):
                    transpose_to(knT[:, st * 128:(st + 1) * 128],
                                 kvu_sb[st][:, h * 256:h * 256 + 128])
                for c in range(8):
                    probsT = ptpool.tile([128, 16 * 256], F32)
                    ntile = 2 * c + 2 if causal else 16
                    for tt in ([2 * c, 2 * c + 1] if causal else [2 * c, 2 * c + 1]):
                        kvlen = 128 * (tt + 1) if causal else S
                        nch = (kvlen + 511) // 512
                        probs = ppool.tile([128, S], F32)
                        denp = spool.tile([128, 4], F32)
                        for kc in range(nch):
                            ncols = min(512, kvlen - kc * 512)
                            ps = psS.tile([128, 512], F32, name="ps")
                            nc.tensor.matmul(ps[:, :ncols],
                                             _r(qT1[h][:, tt * 128:(tt + 1) * 128]),
                                             _r(knT[:, kc * 512:kc * 512 + ncols]),
                                             start=True, stop=False)
                            nc.tensor.matmul(ps[:, :ncols],
                                             _r(qT2[h][:, tt * 128:(tt + 1) * 128]),
                                             _r(kpeT[:, kc * 512:kc * 512 + ncols]),
                                             start=False, stop=True)
                            if causal:
                                if kc == nch - 1:
                                    dcol = tt * 128 - kc * 512
                                    nc.vector.tensor_add(ps[:, dcol:dcol + 128],
                                                         ps[:, dcol:dcol + 128],
                                                         md_sb[:])
                            else:
                                mblk = mpool.tile([128, 512], F32)
                                nc.sync.dma_start(
                                    mblk[:, :ncols],
                                    maskf[tt * 128:(tt + 1) * 128, kc * 512:kc * 512 + ncols])
                                nc.vector.tensor_add(ps[:, :ncols], ps[:, :ncols],
                                                     mblk[:, :ncols])
                            nc.scalar.activation(_r(probs[:, kc * 512:kc * 512 + ncols]),
                                                 ps[:, :ncols],
                                                 mybir.ActivationFunctionType.Exp,
                                                 accum_out=denp[:, kc:kc + 1])
                        den = spool.tile([128, 1], F32)
                        nc.vector.tensor_reduce(den, denp[:, :nch],
                                                mybir.AxisListType.X, mybir.AluOpType.add)
                        recip = spool.tile([128, 1], F32)
                        nc.vector.reciprocal(recip, den)
                        kvcols = 128 * (tt + 1) if causal else S
                        if tt % 2 == 0:
                            nc.vector.tensor_scalar_mul(_r(probs[:, :kvcols]),
                                                        probs[:, :kvcols], recip[:])
                        else:
                            nc.scalar.mul(_r(probs[:, :kvcols]), probs[:, :kvcols],
                                          recip[:])
                        nkt = tt + 1 if causal else 16
                        for kt in range(nkt):
                            dst = probsT[:, kt * 256 + (tt % 2) * 128:kt * 256 + (tt % 2) * 128 + 128]
                            transpose_to(dst, probs[:, kt * 128:(kt + 1) * 128],
                                         copy_eng=nc.vector if kt % 2 == 0 else nc.scalar)
                        if causal and tt % 2 == 1:
                            nc.vector.tensor_copy(out=_r(probsT[:, tt * 256:tt * 256 + 128]),
                                                  in_=zero_sb[:])
                    pso_full = psO.tile([128, 256], F32, name="pso")
                    pso = pso_full[:]
                    for kt in range(ntile):
                        nc.tensor.matmul(pso,
                                         _r(kvu_sb[kt][:, h * 256 + 128:h * 256 + 256]),
                                         _r(probsT[:, kt * 256:(kt + 1) * 256]),
                                         start=(kt == 0), stop=(kt == ntile - 1))
                    nc.scalar.copy(_r(attnT[h][:, c * 256:(c + 1) * 256]), pso)
        qT_pool.release()
        kvu_pool.release()

        # ---------------- Phase G: output projection ----------------------
        with ExitStack() as phG:
            wpool = phG.enter_context(tc.tile_pool(name="wo", bufs=1))
            opool = phG.enter_context(tc.tile_pool(name="osb", bufs=4))
            wo_sb = [wpool.tile([128, D], F32, name=f"wo{k}") for k in range(4)]
            for k in range(4):
                nc.sync.dma_start(_r(wo_sb[k][:]), _r(woT[k * 128:(k + 1) * 128, :]))
            for st in range(16):
                for n in range(4):
                    ps = psB.tile([128, 512], F32, name="ps")
                    for hk in range(4):
                        nc.tensor.matmul(ps[:],
                                         _r(attnT[hk][:, st * 128:(st + 1) * 128]),
                                         _r(wo_sb[hk][:, n * 512:(n + 1) * 512]),
                                         start=(hk == 0), stop=(hk == 3))
                    osb = opool.tile([128, 512], F32)
                    if n % 2 == 0:
                        nc.vector.tensor_copy(out=osb[:], in_=ps[:])
                    else:
                        nc.scalar.copy(osb[:], ps[:])
                    nc.gpsimd.dma_start(
                        out[st * 128:(st + 1) * 128, n * 512:(n + 1) * 512], osb[:])
        attn_pool.release()

    nc.compile()
    return nc


def kernel(x, freqs_cos, freqs_sin, mask, wq_a, q_norm_w, wq_b, wkv_a,
           kv_norm_w, wkv_b, wo, _trace=False):
    global last_exec_time_ns, last_results
    x = np.asarray(x, dtype=np.float32)
    freqs_cos = np.asarray(freqs_cos, dtype=np.float32)
    freqs_sin = np.asarray(freqs_sin, dtype=np.float32)
    mask = np.asarray(mask, dtype=np.float32)

    causal_ref = np.triu(np.full((S, S), -np.inf, dtype=np.float32), k=1)
    causal = bool(np.array_equal(mask, causal_ref))

    if causal not in _cache:
        _cache[causal] = _build(causal)
    nc = _cache[causal]

    scale = QK_D ** -0.5
    wqb_eff = (np.asarray(wq_b, np.float32) * np.asarray(q_norm_w, np.float32)[None, :]
               * scale).astype(np.float32)
    wkvb_eff = (np.asarray(wkv_b, np.float32)
                * np.asarray(kv_norm_w, np.float32)[None, :]).astype(np.float32)
    wqaT = np.ascontiguousarray(np.asarray(wq_a, np.float32).T)
    wkvaT = np.ascontiguousarray(np.asarray(wkv_a, np.float32).T)
    wqbT = np.ascontiguousarray(wqb_eff.T)      # [Q_LORA, H*QK_D]
    wkvbT = np.ascontiguousarray(wkvb_eff.T)    # [KV_LORA, H*256]
    woT_full = np.ascontiguousarray(np.asarray(wo, np.float32).T)  # [H*DV, D]
    cos4 = np.ascontiguousarray(
        np.broadcast_to(freqs_cos[:, None, :], (S, 4, D_ROPE // 2)).reshape(S, -1))
    sin4 = np.ascontiguousarray(
        np.broadcast_to(freqs_sin[:, None, :], (S, 4, D_ROPE // 2)).reshape(S, -1))
    if causal:
        maskd = np.stack([mask[t * 128:(t + 1) * 128, t * 128:(t + 1) * 128]
                          for t in range(16)]).astype(np.float32)

    in_maps = []
    for c in range(N_CORES):
        b, hg = c // 4, c % 4
        sc = c % 4
        im = {
            "xT": np.ascontiguousarray(x[b, sc * SC:(sc + 1) * SC, :].T),
            "wqaT": wqaT, "wkvaT": wkvaT,
            "wqbT": np.ascontiguousarray(wqbT[:, hg * 768:(hg + 1) * 768]),
            "wkvbT": np.ascontiguousarray(wkvbT[:, hg * 1024:(hg + 1) * 1024]),
            "woT": np.ascontiguousarray(woT_full[hg * HDV:(hg + 1) * HDV, :]),
            "cos4": cos4, "sin4": sin4,
            "cosk": np.ascontiguousarray(freqs_cos[sc * SC:(sc + 1) * SC, :]),
            "sink": np.ascontiguousarray(freqs_sin[sc * SC:(sc + 1) * SC, :]),
        }
        if causal:
            im["maskd"] = maskd
        else:
            im["maskf"] = mask
        in_maps.append(im)

    kw = {}
    if _trace:
        kw = dict(trace=True, trace_cores=list(range(N_CORES)))
    res = run_bass_kernel_spmd(nc, in_maps, list(range(N_CORES)), **kw)
    last_exec_time_ns = res.exec_time_ns
    last_results = res
    out = np.zeros((B, S, D), dtype=np.float32)
    for c in range(N_CORES):
        out[c // 4] += res.results[c]["out"]
    return out


# revision 19
# speedup vs baseline: 1.0198x; 1.0198x over previous
"""MLA (multi-head latent attention) Bass kernel for Trainium2, 8 NeuronCores.

Sharding: core c -> batch b=c//4, head group hg=c%4 (4 heads each), plus
sequence-parallel down-projections (seq chunk sc=c%4, 512 rows) with on-device
AllGather of the low-rank latents. Final output projection produces per-core
partial sums over its 4 heads; the host sums the 4 partials per batch.

Host-side prep (sharding/layout only): transpose x chunk and weights so the
contraction dim lands on SBUF partitions, fold RMS-norm weights and the
attention scale into the up-projection weights, extract diagonal mask blocks.
"""
import math
import numpy as np
from contextlib import ExitStack

import concourse.bass as bass
import concourse.tile as tile
from concourse import mybir, bacc
from concourse.bass_utils import run_bass_kernel_spmd

# Problem constants (hardcoded per contract)
B, S, D, H = 2, 2048, 2048, 16
Q_LORA, KV_LORA = 1536, 512
D_NOPE, D_ROPE, D_V = 128, 64, 128
QK_D = D_NOPE + D_ROPE  # 192
HDV = 4 * D_V  # per-core head-group output dim (512)
EPS = 1e-6
N_CORES = 8
SC = S // 4  # seq chunk per core within a batch group (512)
F32 = mybir.dt.float32
F32R = mybir.dt.float32r

_cache = {}
last_exec_time_ns = None
last_results = None


def _r(ap):
    return ap.bitcast(F32R)


def _bcast_free(ap, n):
    """[P,1] AP -> [P,n] free-dim 0-stride broadcast view."""
    return bass.AP(tensor=ap.tensor, offset=ap.offset, ap=[ap.ap[0], [0, n]])


def _build(causal: bool):
    nc = bacc.Bacc(trn_type="TRN2", target_bir_lowering=False, debug=False,
                   num_devices=N_CORES)

    def din(name, shape):
        return nc.dram_tensor(name, shape, F32, kind="ExternalInput").ap()

    xT = din("xT", [D, SC])
    wqaT = din("wqaT", [D, Q_LORA])
    wkvaT = din("wkvaT", [D, KV_LORA + D_ROPE])
    wqbT = din("wqbT", [Q_LORA, 4 * QK_D])
    wkvbT = din("wkvbT", [KV_LORA, 4 * (D_NOPE + D_V)])
    woT = din("woT", [HDV, D])
    cos4 = din("cos4", [S, 4 * (D_ROPE // 2)])  # per-head-replicated cos, [S,128]
    sin4 = din("sin4", [S, 4 * (D_ROPE // 2)])
    cosk = din("cosk", [SC, D_ROPE // 2])  # this core's seq chunk rows
    sink = din("sink", [SC, D_ROPE // 2])
    if causal:
        maskd = din("maskd", [16, 128, 128])  # diagonal blocks of mask
    else:
        maskf = din("maskf", [S, S])
    out = nc.dram_tensor("out", [S, D], F32, kind="ExternalOutput").ap()

    kv_stage = nc.dram_tensor("kv_stage", [KV_LORA + D_ROPE, SC], F32).ap()
    kv_gather = nc.dram_tensor("kv_gather", [4, KV_LORA + D_ROPE, SC], F32).ap()
    cq_stage = nc.dram_tensor("cq_stage", [Q_LORA, SC], F32).ap()
    cq_gather = nc.dram_tensor("cq_gather", [4, Q_LORA, SC], F32).ap()
    GROUPS = [[0, 1, 2, 3], [4, 5, 6, 7]]

    with tile.TileContext(nc) as tc, ExitStack() as top:
        persist = top.enter_context(tc.tile_pool(name="persist", bufs=1))
        ident0 = persist.tile([128, 128], F32)
        nc.gpsimd.memset(ident0[:], 0.0)
        nc.gpsimd.affine_select(
            out=ident0[:], in_=ident0[:],
            compare_op=mybir.AluOpType.not_equal, fill=1.0,
            base=0, pattern=[[-1, 128]], channel_multiplier=1)
        ident = persist.tile([128, 128], F32)
        nc.vector.tensor_copy(out=_r(ident[:]), in_=ident0[:])
        eps_sb = persist.tile([128, 1], F32)
        nc.vector.memset(eps_sb, EPS)
        zero_sb = persist.tile([128, 128], F32)
        nc.vector.memset(zero_sb, 0.0)

        psB = top.enter_context(tc.tile_pool(name="psB", bufs=2, space="PSUM"))
        psT = top.enter_context(tc.tile_pool(name="psT", bufs=3, space="PSUM"))

        def rms_norm(out_ap, in_ap, ddim, tmp_pool):
            sq = tmp_pool.tile([128, ddim], F32)
            nc.vector.tensor_mul(sq, in_ap, in_ap)
            ss = tmp_pool.tile([128, 1], F32)
            nc.vector.tensor_reduce(ss, sq, mybir.AxisListType.X, mybir.AluOpType.add)
            std = tmp_pool.tile([128, 1], F32)
            nc.scalar.activation(std, ss, mybir.ActivationFunctionType.Sqrt,
                                 bias=eps_sb, scale=1.0 / ddim)
            rstd = tmp_pool.tile([128, 1], F32)
            nc.vector.reciprocal(rstd, std)
            nc.scalar.mul(_r(out_ap), in_ap, rstd)

        def rope(out3, in3, cos_ap, sin_ap, nh, tmp_pool):
            # in3/out3: [128, nh, 64] views (pairs interleaved in last dim);
            # cos/sin: [128, nh*32] contiguous tiles. Safe for out3 == in3.
            def iv(a3, par):  # [128, nh, 32] view of pair element par
                r2 = a3.rearrange("p h (d two) -> p h d two", two=2)
                return r2[:, :, :, par]
            c3 = cos_ap.rearrange("p (h d) -> p h d", h=nh)
            s3 = sin_ap.rearrange("p (h d) -> p h d", h=nh)
            xr, xi = iv(in3, 0), iv(in3, 1)
            t1 = tmp_pool.tile([128, nh, 32], F32)
            t2 = tmp_pool.tile([128, nh, 32], F32)
            t3 = tmp_pool.tile([128, nh, 32], F32)
            t4 = tmp_pool.tile([128, nh, 32], F32)
            nc.vector.tensor_mul(t1, xr, c3)
            nc.vector.tensor_mul(t2, xi, s3)
            nc.vector.tensor_mul(t3, xr, s3)
            nc.vector.tensor_mul(t4, xi, c3)
            nc.vector.tensor_sub(_r(iv(out3, 0)), t1, t2)
            nc.vector.tensor_add(_r(iv(out3, 1)), t3, t4)

        def transpose_to(dst_ap, src_ap, rhs=None, dt_r=True, copy_eng=None):
            # PE transpose src [p,f] -> psum [f,p] (f32r), copy into dst_ap
            f = src_ap.shape[1]
            ps = psT.tile([128, 128], F32, name="ps")
            nc.tensor.matmul(_r(ps[:f, :src_ap.shape[0]]), _r(src_ap),
                             _r(ident[:] if rhs is None else rhs),
                             is_transpose=True)
            eng = copy_eng or nc.vector
            if eng is nc.scalar:
                eng.copy(_r(dst_ap), _r(ps[:f, :src_ap.shape[0]]))
            else:
                eng.tensor_copy(out=_r(dst_ap), in_=_r(ps[:f, :src_ap.shape[0]]))

        # ---------------- Phase A: load xT ----------------
        xT_pool = tc.alloc_tile_pool(name="xT", bufs=1)
        xT_sb = []
        for k in range(16):
            t = xT_pool.tile([128, SC], F32, name=f"xT{k}")
            nc.sync.dma_start(_r(t[:]), _r(xT[k * 128:(k + 1) * 128, :]))
            xT_sb.append(t)

        # ---------------- Phase B: kv down-proj + norm + rope + T + AG ----
        with ExitStack() as phB:
            wpool = phB.enter_context(tc.tile_pool(name="wkva", bufs=2))
            kvf_pool = phB.enter_context(tc.tile_pool(name="kvf", bufs=1))
            tmp = phB.enter_context(tc.tile_pool(name="tmpB", bufs=4))
            stg = phB.enter_context(tc.tile_pool(name="stgB", bufs=4))
            kvf_sb = [kvf_pool.tile([128, KV_LORA + D_ROPE], F32, name=f"kvf{i}") for i in range(4)]
            for (n0, nw) in [(0, 288), (288, 288)]:
                wk = [wpool.tile([128, nw], F32, name=f"wkva_{k}") for k in range(16)]
                for k in range(16):
                    nc.sync.dma_start(_r(wk[k][:]), _r(wkvaT[k * 128:(k + 1) * 128, n0:n0 + nw]))
                for stl in range(4):
                    ps = psB.tile([128, 512], F32, name="ps")
                    for k in range(16):
                        nc.tensor.matmul(ps[:, :nw], _r(xT_sb[k][:, stl * 128:(stl + 1) * 128]),
                                         _r(wk[k][:]), start=(k == 0), stop=(k == 15))
                    eng = nc.vector if stl % 2 == 0 else nc.scalar
                    if eng is nc.vector:
                        nc.vector.tensor_copy(out=_r(kvf_sb[stl][:, n0:n0 + nw]), in_=ps[:, :nw])
                    else:
                        nc.scalar.copy(_r(kvf_sb[stl][:, n0:n0 + nw]), ps[:, :nw])
            for stl in range(4):
                rms_norm(kvf_sb[stl][:, :KV_LORA], kvf_sb[stl][:, :KV_LORA], KV_LORA, tmp)
                ck = tmp.tile([128, 32], F32)
                sk = tmp.tile([128, 32], F32)
                nc.sync.dma_start(ck[:], cosk[stl * 128:(stl + 1) * 128, :])
                nc.sync.dma_start(sk[:], sink[stl * 128:(stl + 1) * 128, :])
                kpe = tmp.tile([128, D_ROPE], F32)
                rope(kpe[:].rearrange("p (h d) -> p h d", h=1),
                     kvf_sb[stl][:, KV_LORA:].rearrange("p (h d) -> p h d", h=1),
                     ck[:], sk[:], 1, tmp)
                for dt_ in range(4):
                    blk = stg.tile([128, 128], F32)
                    transpose_to(blk[:], kvf_sb[stl][:, dt_ * 128:(dt_ + 1) * 128])
                    nc.gpsimd.dma_start(
                        kv_stage[dt_ * 128:(dt_ + 1) * 128, stl * 128:(stl + 1) * 128], blk[:])
                blk = stg.tile([64, 128], F32)
                transpose_to(blk[:], kpe[:])
                nc.gpsimd.dma_start(
                    kv_stage[KV_LORA:, stl * 128:(stl + 1) * 128], blk[:])
            nc.gpsimd.collective_compute(
                "AllGather", mybir.AluOpType.bypass, replica_groups=GROUPS,
                ins=[kv_stage[:]], outs=[kv_gather[:]])

        # ---------------- Phase C: cq down-proj + norm + T + AG ----------
        with ExitStack() as phC:
            wpool = phC.enter_context(tc.tile_pool(name="wqa", bufs=2))
            cq_pool = phC.enter_context(tc.tile_pool(name="cq", bufs=1))
            tmp = phC.enter_context(tc.tile_pool(name="tmpC", bufs=4))
            stg = phC.enter_context(tc.tile_pool(name="stgC", bufs=4))
            cq_sb = [cq_pool.tile([128, Q_LORA], F32, name=f"cqsb{i}") for i in range(4)]
            for ci in range(3):
                n0 = ci * 512
                wk = [wpool.tile([128, 512], F32, name=f"wqa_{k}") for k in range(16)]
                for k in range(16):
                    nc.sync.dma_start(_r(wk[k][:]), _r(wqaT[k * 128:(k + 1) * 128, n0:n0 + 512]))
                for stl in range(4):
                    ps = psB.tile([128, 512], F32, name="ps")
                    for k in range(16):
                        nc.tensor.matmul(ps[:], _r(xT_sb[k][:, stl * 128:(stl + 1) * 128]),
                                         _r(wk[k][:]), start=(k == 0), stop=(k == 15))
                    if stl % 2 == 0:
                        nc.vector.tensor_copy(out=_r(cq_sb[stl][:, n0:n0 + 512]), in_=ps[:])
                    else:
                        nc.scalar.copy(_r(cq_sb[stl][:, n0:n0 + 512]), ps[:])
            for stl in range(4):
                rms_norm(cq_sb[stl][:], cq_sb[stl][:], Q_LORA, tmp)
                for dt_ in range(12):
                    blk = stg.tile([128, 128], F32)
                    transpose_to(blk[:], cq_sb[stl][:, dt_ * 128:(dt_ + 1) * 128])
                    nc.gpsimd.dma_start(
                        cq_stage[dt_ * 128:(dt_ + 1) * 128, stl * 128:(stl + 1) * 128], blk[:])
            nc.gpsimd.collective_compute(
                "AllGather", mybir.AluOpType.bypass, replica_groups=GROUPS,
                ins=[cq_stage[:]], outs=[cq_gather[:]])
        xT_pool.release()

        # ---------------- Phase D: kv up-proj (full S, this head group) ---
        kvu_pool = tc.alloc_tile_pool(name="kvu", bufs=1, side="right")
        kvu_sb = [kvu_pool.tile([128, 1024], F32, name=f"kvu{st}") for st in range(16)]
        with ExitStack() as phD:
            wpool = phD.enter_context(tc.tile_pool(name="wkvb", bufs=1))
            lpool = phD.enter_context(tc.tile_pool(name="kvl", bufs=3))
            wb = [wpool.tile([128, 1024], F32, name=f"wkvb{k}") for k in range(4)]
            for k in range(4):
                nc.sync.dma_start(_r(wb[k][:]), _r(wkvbT[k * 128:(k + 1) * 128, :]))
            for st in range(16):
                g, stl = st // 4, st % 4
                lk = [lpool.tile([128, 128], F32, name=f"kvlk{k}") for k in range(4)]
                for k in range(4):
                    nc.sync.dma_start(
                        _r(lk[k][:]), _r(kv_gather[g, k * 128:(k + 1) * 128,
                                                   stl * 128:(stl + 1) * 128]))
                for ncho in range(2):
                    ps = psB.tile([128, 512], F32, name="ps")
                    for k in range(4):
                        nc.tensor.matmul(ps[:], _r(lk[k][:]),
                                         _r(wb[k][:, ncho * 512:(ncho + 1) * 512]),
                                         start=(k == 0), stop=(k == 3))
                    if (st + ncho) % 2 == 0:
                        nc.vector.tensor_copy(out=_r(kvu_sb[st][:, ncho * 512:(ncho + 1) * 512]), in_=ps[:])
                    else:
                        nc.scalar.copy(_r(kvu_sb[st][:, ncho * 512:(ncho + 1) * 512]), ps[:])

        # ---------------- Phase E: q up-proj + rope + qT ------------------
        qT_pool = tc.alloc_tile_pool(name="qT", bufs=1, side="right")
        qT1 = [qT_pool.tile([128, S], F32, name=f"qT1_{h}") for h in range(4)]
        qT2 = [qT_pool.tile([64, S], F32, name=f"qT2_{h}") for h in range(4)]
        with ExitStack() as phE:
            wpool = phE.enter_context(tc.tile_pool(name="wqb", bufs=1))
            lpool = phE.enter_context(tc.tile_pool(name="cql", bufs=2))
            qpool = phE.enter_context(tc.tile_pool(name="qsb", bufs=3))
            tmp = phE.enter_context(tc.tile_pool(name="tmpE", bufs=4))
            wb = [wpool.tile([128, 768], F32, name=f"wqb{k}") for k in range(12)]
            for k in range(12):
                nc.sync.dma_start(_r(wb[k][:]), _r(wqbT[k * 128:(k + 1) * 128, :]))
            for st in range(16):
                g, stl = st // 4, st % 4
                lk = [lpool.tile([128, 128], F32, name=f"cqlk{k}") for k in range(12)]
                for k in range(12):
                    nc.sync.dma_start(
                        _r(lk[k][:]), _r(cq_gather[g, k * 128:(k + 1) * 128,
                                                   stl * 128:(stl + 1) * 128]))
                q_sb = qpool.tile([128, 768], F32)
                for (n0, nw) in [(0, 512), (512, 256)]:
                    ps = psB.tile([128, 512], F32, name="ps")
                    for k in range(12):
                        nc.tensor.matmul(ps[:, :nw], _r(lk[k][:]),
                                         _r(wb[k][:, n0:n0 + nw]),
                                         start=(k == 0), stop=(k == 11))
                    if n0 == 0:
                        nc.vector.tensor_copy(out=_r(q_sb[:, :512]), in_=ps[:, :512])
                    else:
                        nc.scalar.copy(_r(q_sb[:, 512:]), ps[:, :256])
                c4 = tmp.tile([128, 128], F32)
                s4 = tmp.tile([128, 128], F32)
                nc.sync.dma_start(c4[:], cos4[st * 128:(st + 1) * 128, :])
                nc.sync.dma_start(s4[:], sin4[st * 128:(st + 1) * 128, :])
                # rope the pe sub-blocks of the 4 heads: cols h*192+128 .. +64
                qpe = q_sb[:].rearrange("p (h d) -> p h d", h=4)[:, :, D_NOPE:]
                rope(qpe, qpe, c4[:], s4[:], 4, tmp)
                for hh in range(4):
                    transpose_to(qT1[hh][:, st * 128:(st + 1) * 128],
                                 q_sb[:, hh * 192:hh * 192 + 128])
                    transpose_to(qT2[hh][:, st * 128:(st + 1) * 128],
                                 q_sb[:, hh * 192 + 128:hh * 192 + 192])

        # ---------------- Phase F: attention per head ---------------------
        attn_pool = tc.alloc_tile_pool(name="attnT", bufs=1)
        attnT = [attn_pool.tile([128, S], F32, name=f"attnT{h}") for h in range(4)]
        with ExitStack() as phF:
            kpool = phF.enter_context(tc.tile_pool(name="knT", bufs=1))
            ppool = phF.enter_context(tc.tile_pool(name="probs", bufs=1))
            ptpool = phF.enter_context(tc.tile_pool(name="probsT", bufs=1))
            spool = phF.enter_context(tc.tile_pool(name="smallF", bufs=8))
            mpool = phF.enter_context(tc.tile_pool(name="maskp", bufs=1 if causal else 6))
            psS = phF.enter_context(tc.tile_pool(name="psS", bufs=2, space="PSUM"))
            psO = phF.enter_context(tc.tile_pool(name="psO", bufs=1, space="PSUM"))
            kpeT = kpool.tile([64, S], F32)
            for g in range(4):
                nc.sync.dma_start(_r(kpeT[:, g * 512:(g + 1) * 512]),
                                  _r(kv_gather[g, KV_LORA:, :]))
            if causal:
                # all 16 diagonal blocks of a causal mask are identical
                md_sb = mpool.tile([128, 128], F32, name="md0")
                nc.sync.dma_start(md_sb[:], maskd[0])
            knT = kpool.tile([128, S], F32)
            for h in range(4):
                for st in range(16):
                    transpose_to(knT[:, st * 128:(st + 1) * 128],
                                 kvu_sb[st][:, h * 256:h * 256 + 128])
                for c in range(8):
                    probsT = ptpool.tile([128, 16 * 256], F32)
                    ntile = 2 * c + 2 if causal else 16
                    for tt in ([2 * c, 2 * c + 1] if causal else [2 * c, 2 * c + 1]):
                        kvlen = 128 * (tt + 1) if causal else S
                        nch = (kvlen + 511) // 512
                        probs = ppool.tile([128, S], F32)
                        denp = spool.tile([128, 4], F32)
                        for kc in range(nch):
                            ncols = min(512, kvlen - kc * 512)
                            ps = psS.tile([128, 512], F32, name="ps")
                            nc.tensor.matmul(ps[:, :ncols],
                                             _r(qT1[h][:, tt * 128:(tt + 1) * 128]),
                                             _r(knT[:, kc * 512:kc * 512 + ncols]),
                                             start=True, stop=False)
                            nc.tensor.matmul(ps[:, :ncols],
                                             _r(qT2[h][:, tt * 128:(tt + 1) * 128]),
                                             _r(kpeT[:, kc * 512:kc * 512 + ncols]),
                                             start=False, stop=True)
                            if causal:
                                if kc == nch - 1:
                                    dcol = tt * 128 - kc * 512
                                    nc.vector.tensor_add(ps[:, dcol:dcol + 128],
                                                         ps[:, dcol:dcol + 128],
                                                         md_sb[:])
                            else:
                                mblk = mpool.tile([128, 512], F32)
                                nc.sync.dma_start(
                                    mblk[:, :ncols],
                                    maskf[tt * 128:(tt + 1) * 128, kc * 512:kc * 512 + ncols])
                                nc.vector.tensor_add(ps[:, :ncols], ps[:, :ncols],
                                                     mblk[:, :ncols])
                            nc.scalar.activation(_r(probs[:, kc * 512:kc * 512 + ncols]),
                                                 ps[:, :ncols],
                                                 mybir.ActivationFunctionType.Exp,
                                                 accum_out=denp[:, kc:kc + 1])
                        den = spool.tile([128, 1], F32)
                        nc.vector.tensor_reduce(den, denp[:, :nch],
                                                mybir.AxisListType.X, mybir.AluOpType.add)
                        recip = spool.tile([128, 1], F32)
                        nc.vector.reciprocal(recip, den)
                        kvcols = 128 * (tt + 1) if causal else S
                        if tt % 2 == 0:
                            nc.vector.tensor_scalar_mul(_r(probs[:, :kvcols]),
                                                        probs[:, :kvcols], recip[:])
                        else:
                            nc.scalar.mul(_r(probs[:, :kvcols]), probs[:, :kvcols],
                                          recip[:])
                        nkt = tt + 1 if causal else 16
                        for kt in range(nkt):
                            dst = probsT[:, kt * 256 + (tt % 2) * 128:kt * 256 + (tt % 2) * 128 + 128]
                            transpose_to(dst, probs[:, kt * 128:(kt + 1) * 128],
                                         copy_eng=nc.vector if kt % 2 == 0 else nc.scalar)
                        if causal and tt % 2 == 1:
                            nc.vector.tensor_copy(out=_r(probsT[:, tt * 256:tt * 256 + 128]),
                                                  in_=zero_sb[:])
                    pso_full = psO.tile([128, 256], F32, name="pso")
                    pso = pso_full[:]
                    for kt in range(ntile):
                        nc.tensor.matmul(pso,
                                         _r(kvu_sb[kt][:, h * 256 + 128:h * 256 + 256]),
                                         _r(probsT[:, kt * 256:(kt + 1) * 256]),
                                         start=(kt == 0), stop=(kt == ntile - 1))
                    nc.scalar.copy(_r(attnT[h][:, c * 256:(c + 1) * 256]), pso)
        qT_pool.release()
        kvu_pool.release()

        # ---------------- Phase G: output projection ----------------------
        with ExitStack() as phG:
            wpool = phG.enter_context(tc.tile_pool(name="wo", bufs=1))
            opool = phG.enter_context(tc.tile_pool(name="osb", bufs=4))
            wo_sb = [wpool.tile([128, D], F32, name=f"wo{k}") for k in range(4)]
            for k in range(4):
                nc.sync.dma_start(_r(wo_sb[k][:]), _r(woT[k * 128:(k + 1) * 128, :]))
            for st in range(16):
                for n in range(4):
                    ps = psB.tile([128, 512], F32, name="ps")
                    for hk in range(4):
                        nc.tensor.matmul(ps[:],
                                         _r(attnT[hk][:, st * 128:(st + 1) * 128]),
                                         _r(wo_sb[hk][:, n * 512:(n + 1) * 512]),
                                         start=(hk == 0), stop=(hk == 3))
                    osb = opool.tile([128, 512], F32)
                    if n % 2 == 0:
                        nc.vector.tensor_copy(out=osb[:], in_=ps[:])
                    else:
                        nc.scalar.copy(osb[:], ps[:])
                    nc.gpsimd.dma_start(
                        out[st * 128:(st + 1) * 128, n * 512:(n + 1) * 512], osb[:])
        attn_pool.release()

    nc.compile()
    return nc


def kernel(x, freqs_cos, freqs_sin, mask, wq_a, q_norm_w, wq_b, wkv_a,
           kv_norm_w, wkv_b, wo, _trace=False):
    global last_exec_time_ns, last_results
    x = np.asarray(x, dtype=np.float32)
    freqs_cos = np.asarray(freqs_cos, dtype=np.float32)
    freqs_sin = np.asarray(freqs_sin, dtype=np.float32)
    mask = np.asarray(mask, dtype=np.float32)

    causal_ref = np.triu(np.full((S, S), -np.inf, dtype=np.float32), k=1)
    causal = bool(np.array_equal(mask, causal_ref))

    if causal not in _cache:
        _cache[causal] = _build(causal)
    nc = _cache[causal]

    scale = QK_D ** -0.5
    wqb_eff = (np.asarray(wq_b, np.float32) * np.asarray(q_norm_w, np.float32)[None, :]
               * scale).astype(np.float32)
    wkvb_eff = (np.asarray(wkv_b, np.float32)
                * np.asarray(kv_norm_w, np.float32)[None, :]).astype(np.float32)
    wqaT = np.ascontiguousarray(np.asarray(wq_a, np.float32).T)
    wkvaT = np.ascontiguousarray(np.asarray(wkv_a, np.float32).T)
    wqbT = np.ascontiguousarray(wqb_eff.T)      # [Q_LORA, H*QK_D]
    wkvbT = np.ascontiguousarray(wkvb_eff.T)    # [KV_LORA, H*256]
    woT_full = np.ascontiguousarray(np.asarray(wo, np.float32).T)  # [H*DV, D]
    cos4 = np.ascontiguousarray(
        np.broadcast_to(freqs_cos[:, None, :], (S, 4, D_ROPE // 2)).reshape(S, -1))
    sin4 = np.ascontiguousarray(
        np.broadcast_to(freqs_sin[:, None, :], (S, 4, D_ROPE // 2)).reshape(S, -1))
    if causal:
        maskd = np.stack([mask[t * 128:(t + 1) * 128, t * 128:(t + 1) * 128]
                          for t in range(16)]).astype(np.float32)

    in_maps = []
    for c in range(N_CORES):
        b, hg = c // 4, c % 4
        sc = c % 4
        im = {
            "xT": np.ascontiguousarray(x[b, sc * SC:(sc + 1) * SC, :].T),
            "wqaT": wqaT, "wkvaT": wkvaT,
            "wqbT": np.ascontiguousarray(wqbT[:, hg * 768:(hg + 1) * 768]),
            "wkvbT": np.ascontiguousarray(wkvbT[:, hg * 1024:(hg + 1) * 1024]),
            "woT": np.ascontiguousarray(woT_full[hg * HDV:(hg + 1) * HDV, :]),
            "cos4": cos4, "sin4": sin4,
            "cosk": np.ascontiguousarray(freqs_cos[sc * SC:(sc + 1) * SC, :]),
            "sink": np.ascontiguousarray(freqs_sin[sc * SC:(sc + 1) * SC, :]),
        }
        if causal:
            im["maskd"] = maskd
        else:
            im["maskf"] = mask
        in_maps.append(im)

    kw = {}
    if _trace:
        kw = dict(trace=True, trace_cores=list(range(N_CORES)))
    res = run_bass_kernel_spmd(nc, in_maps, list(range(N_CORES)), **kw)
    last_exec_time_ns = res.exec_time_ns
    last_results = res
    out = np.zeros((B, S, D), dtype=np.float32)
    for c in range(N_CORES):
        out[c // 4] += res.results[c]["out"]
    return out
